# revision 18
# baseline (speedup 1.0000x reference)
"""Trainium2 Bass kernel for the MFA/MPPCA mixture log-likelihood problem.

Math: out[n,k] = PI[k] + logprob[n,k] with Sigma_k = A_k A_k^T + diag(D_k^2),
computed via Woodbury.  Everything involving only the small parameters
(MU, A, D, PI) is folded on the host into:

    out[n,k] = CONST[k] + x[n]·H[:,k] + (x[n]^2)·G[:,k] + sum_l (x[n]·Csc[:,k,l])^2

where (with iD = D^-2, B = iD*A, L = I + A^T B, iL = inv(L), R = chol(iL),
C0 = B R, e = R^T B^T MU):
    G   = -0.5 * iD^T                       (d, K)
    H   = (iD*MU)^T - C0 e                  (d, K)
    Csc = sqrt(0.5) * C0                    (d, K*l)
    CONST = PI - 0.5*(d log 2pi + logdet Sigma + MU^T iD MU) + 0.5 |e|^2

Device kernel (data-parallel over N on 8 cores, fp16 weights/inputs with
fp32 PSUM): x and x^2 are packed on the host into one fp16 stream per
128-sample tile (8 chunks of 128 features) so each tile needs exactly one
2KB-per-partition DMA line.  PE accumulates [H+G | Csc] projections into a
2-psum-bank tile (region a = H + components 0:32 at cols 0:384, region b =
components 32:64 at cols 512:832).  One ScalarE ACTIVATE squares all 640
factor projections (4-dim AP over both regions) into a fp32 [128, 64, 12]
buffer whose lane 10 is pre-filled with CONST[k] and lane 11 receives the
psum H+G block via a ScalarE copy; a single VectorE group-of-12 reduce then
emits the finished output row (q2 + CONST + H + G) directly.  DMAs are
batched 4 tiles at a time; a garbage-matmul warmup spans the initial DMA
window so the PE HAM clock-gate never starts the real work cold.
"""
import math
import numpy as np

N_TOTAL, K, D_FEAT, L_FAC = 131072, 64, 512, 10
N_CORES = 8
N_PER_CORE = N_TOTAL // N_CORES  # 16384

KHALF = K // 2                     # 32 components per psum region
ACOL = K + KHALF * L_FAC           # 384 = [H (0:64) | comps 0:32 (64:384)]
BCOL = KHALF * L_FAC               # 320 = comps 32:64
BOFF = 512                         # psum col offset of region b (bank 1)
TILES = N_PER_CORE // 128          # 128
DMA_BATCH = 4                      # sample tiles per input/output DMA


def host_prep(MU, A, D, PI):
    """Fold small-parameter math into matmul weights (float64 internally)."""
    MU64, A64, D64, PI64 = [np.asarray(v, np.float64) for v in (MU, A, D, PI)]
    Kc, d, l = A64.shape
    iD = D64 ** -2.0
    B = iD[..., None] * A64
    L = np.eye(l)[None] + np.einsum('kdl,kdm->klm', A64, B)
    sign, logdet_L = np.linalg.slogdet(L)
    log_det_Sigma = logdet_L - np.sum(np.log(iD), axis=1)
    iL = np.linalg.inv(L)
    R = np.linalg.cholesky(iL)                  # R @ R.T = iL
    C0 = np.einsum('kdl,klm->kdm', B, R)        # (K, d, l)
    bmu = np.einsum('kdl,kd->kl', B, MU64)
    e = np.einsum('klm,kl->km', R, bmu)         # (K, l)
    c1 = np.sum(iD * MU64 * MU64, axis=1)

    CONST = PI64 - 0.5 * (d * math.log(2.0 * math.pi) + log_det_Sigma + c1) \
        + 0.5 * np.sum(e * e, axis=1)
    G = (-0.5 * iD).T
    H = (iD * MU64 - np.einsum('kdm,km->kd', C0, e)).T
    Csc = (C0 * np.sqrt(0.5)).transpose(1, 0, 2).reshape(d, Kc * l)  # k-major

    wall = np.concatenate([H, Csc], axis=1).astype(np.float16)     # (d, 704)
    g16 = G.astype(np.float16)                                      # (d, K)
    cline = np.tile(CONST.astype(np.float32)[None, :], (128, 1))    # (128, K)
    return wall, g16, cline


def pack_core_input(xs):
    """xs: (n_per_core, 512) fp32 -> (TILES, 128*8*128) fp16 tile-major pack.

    Per tile t, partition p: 8 chunks of 128 contiguous fp16 values:
    chunks 0:4 = x[d = c*128+p, t*128:(t+1)*128], chunks 4:8 = x^2 likewise.
    """
    n = xs.shape[0]
    x16 = xs.T.astype(np.float16)                      # (512, n)
    x2 = (x16 * x16).astype(np.float16)                # exact squares of fp16 x
    xr = x16.reshape(4, 128, n // 128, 128).transpose(2, 1, 0, 3)
    x2r = x2.reshape(4, 128, n // 128, 128).transpose(2, 1, 0, 3)
    packed = np.concatenate([xr, x2r], axis=2)         # (t, 128, 8, 128)
    return np.ascontiguousarray(packed).reshape(n // 128, 128 * 8 * 128)


def build_nc(n_per_core=N_PER_CORE):
    """Build and compile the Bass module for one core (SPMD across 8)."""
    import concourse.bacc as bacc
    import concourse.tile as tile
    import concourse.mybir as mybir

    f32 = mybir.dt.float32
    f16 = mybir.dt.float16
    assert n_per_core % (128 * DMA_BATCH) == 0
    tiles = n_per_core // 128
    nbatch = tiles // DMA_BATCH

    nc = bacc.Bacc("TRN2", target_bir_lowering=False, debug=False,
                   enable_asserts=False, num_devices=N_CORES)
    xx2_dram = nc.dram_tensor("xx2", (tiles, 128 * 8 * 128), f16,
                              kind="ExternalInput")
    wall_dram = nc.dram_tensor("wall", (D_FEAT, K + K * L_FAC), f16,
                               kind="ExternalInput")
    g_dram = nc.dram_tensor("g16", (D_FEAT, K), f16, kind="ExternalInput")
    c_dram = nc.dram_tensor("cline", (128, K), f32, kind="ExternalInput")
    out_dram = nc.dram_tensor("out", (n_per_core, K), f32, kind="ExternalOutput")

    xx2_v = xx2_dram.ap().rearrange("t (p c j) -> t p c j", p=128, c=8)
    wall_v = wall_dram.ap().rearrange("(c p) m -> p c m", p=128)  # [128, 4, 704]
    g_v = g_dram.ap().rearrange("(c p) m -> p c m", p=128)        # [128, 4, 64]
    out_v = out_dram.ap().rearrange("(b u p) k -> b p u k", p=128, u=DMA_BATCH)

    with tile.TileContext(nc) as tc:
        with (
            tc.tile_pool(name="wpool", bufs=1) as wpool,
            tc.tile_pool(name="xpool", bufs=4) as xpool,
            tc.tile_pool(name="spool", bufs=3) as spool,
            tc.tile_pool(name="opool", bufs=2) as opool,
            tc.tile_pool(name="ppool", bufs=3, space="PSUM") as ppool,
            tc.tile_pool(name="wmpool", bufs=1, space="PSUM") as wmpool,
        ):
            # --- HAM warmup: keep PE busy at 2.4GHz while the first DMAs land.
            # memset on VectorE (fastest engine spin-up ~2.6us); the garbage
            # matmul results go to a scratch psum bank that is never read.
            warm = wpool.tile([128, 512], f16)
            nc.vector.memset(warm[:], 0.0)
            wpsum = wmpool.tile([128, 512], f32)
            NWARM = 14
            for j in range(NWARM):
                nc.tensor.matmul(wpsum[:], warm[:, 0:128], warm[:],
                                 start=(j == 0), stop=(j == NWARM - 1))

            # params go out on the ScalarE hardware-DGE queue so they
            # overlap with the x-batch stream on the sync queue
            wall_sb = wpool.tile([128, 4, K + K * L_FAC], f16)
            nc.scalar.dma_start(out=wall_sb[:], in_=wall_v[:])
            g_sb = wpool.tile([128, 4, K], f16)
            nc.scalar.dma_start(out=g_sb[:], in_=g_v[:])
            c_sb = wpool.tile([128, K], f32)
            nc.scalar.dma_start(out=c_sb[:], in_=c_dram.ap())

            # ping-pong squares buffers [128, 64, 12] fp32: lanes 0:10 get the
            # squared projections, lane 10 holds CONST[k], lane 11 gets the
            # psum H+G block per tile -- the group reduce then emits the
            # finished output row directly
            sq0 = wpool.tile([128, K, 12], f32)
            sq1 = wpool.tile([128, K, 12], f32)
            for s in (sq0, sq1):
                nc.vector.tensor_copy(s[:, :, 10], c_sb[:])
            sqs = (sq0, sq1)

            for b in range(nbatch):
                xb = xpool.tile([128, DMA_BATCH, 8, 128], f16, tag="xb")
                nc.sync.dma_start(out=xb[:], in_=xx2_v[b * DMA_BATCH:
                                                       (b + 1) * DMA_BATCH]
                                  .rearrange("t p c j -> p t c j"))
                ob = opool.tile([128, DMA_BATCH, K], f32, tag="ob")
                for u in range(DMA_BATCH):
                    t = b * DMA_BATCH + u
                    psum = ppool.tile([128, 960], f32, tag="ps")

                    def mm_a(c, start, stop):
                        nc.tensor.matmul(psum[:, 0:ACOL], xb[:, u, c, :],
                                         wall_sb[:, c, 0:ACOL],
                                         start=start, stop=stop)

                    def mm_b(c, start, stop):
                        nc.tensor.matmul(psum[:, BOFF:BOFF + BCOL], xb[:, u, c, :],
                                         wall_sb[:, c, ACOL:ACOL + BCOL],
                                         start=start, stop=stop)

                    mm_a(0, True, False)
                    mm_b(0, True, False)
                    mm_a(1, False, False)
                    mm_b(1, False, False)
                    mm_a(2, False, False)
                    mm_b(2, False, False)
                    for c in range(4):
                        nc.tensor.matmul(psum[:, 0:K], xb[:, u, 4 + c, :],
                                         g_sb[:, c, :], start=False, stop=False,
                                         skip_group_check=True)
                    mm_a(3, False, True)
                    mm_b(3, False, True)

                    # one ACTIVATE squares both regions: AP [128, 2, 32, 10]
                    sq = sqs[t % 2]
                    # regions at cols 64 and 512 -> r-stride 448 cols
                    psq = psum[:, K:K + 2 * (BOFF - K)] \
                        .rearrange("p (r m) -> p r m", r=2)
                    nc.scalar.square(
                        sq[:].rearrange("p (r g) l -> p r g l", r=2)[:, :, :, 0:L_FAC],
                        psq[:, :, 0:KHALF * L_FAC]
                        .rearrange("p r (g t) -> p r g t", t=L_FAC))
                    nc.scalar.copy(sq[:, :, 11], psum[:, 0:K])

                    nc.vector.reduce_sum(ob[:, u, :], sq[:],
                                         axis=mybir.AxisListType.X)

                nc.sync.dma_start(out=out_v[b], in_=ob[:])

    nc.compile()
    return nc


_NC_CACHE = {}


def _get_nc(n_per_core=N_PER_CORE):
    if n_per_core not in _NC_CACHE:
        _NC_CACHE[n_per_core] = build_nc(n_per_core)
    return _NC_CACHE[n_per_core]


def _install_ntff_hook():
    """Provide the antenv.axon_hooks shim so trace=True can capture NTFFs."""
    import sys
    if "antenv.axon_hooks" in sys.modules:
        return
    import types
    import ctypes
    import contextlib

    so_path = "/opt/axon/libaxon_pjrt.so"
    lib = ctypes.CDLL(so_path)
    if not hasattr(lib, "axon_start_nrt_profile"):
        return
    lib.axon_start_nrt_profile.argtypes = [ctypes.POINTER(ctypes.c_int64), ctypes.c_size_t]
    lib.axon_start_nrt_profile.restype = ctypes.c_int64
    lib.axon_stop_nrt_profile.argtypes = [ctypes.c_char_p]
    lib.axon_stop_nrt_profile.restype = ctypes.c_int64

    @contextlib.contextmanager
    def _hook(output_dir, device_ids):
        import jax
        jax.devices()
        if device_ids:
            ids = (ctypes.c_int64 * len(device_ids))(*device_ids)
            rc = lib.axon_start_nrt_profile(ids, len(device_ids))
        else:
            rc = lib.axon_start_nrt_profile(None, 0)
        if rc != 0:
            raise RuntimeError(f"axon_start_nrt_profile rc={rc}")
        try:
            yield
        finally:
            n = lib.axon_stop_nrt_profile(str(output_dir).encode())
            print(f"ntff profile: {n} file(s) written to {output_dir}")

    mod = types.ModuleType("antenv.axon_hooks")
    mod.get_axon_ntff_profile_hook = lambda: _hook
    mod.set_axon_ntff_profile_hook = lambda h: None
    sys.modules["antenv.axon_hooks"] = mod


def kernel(x, MU, A, D, PI, trace=False):
    from concourse.bass_utils import run_bass_kernel_spmd
    if trace:
        try:
            _install_ntff_hook()
        except Exception as e:
            print(f"ntff hook install failed: {e}")
            trace = False

    x = np.asarray(x)
    wall, g16, cline = host_prep(MU, A, D, PI)
    nc = _get_nc()

    in_maps = []
    for c in range(N_CORES):
        packed = pack_core_input(x[c * N_PER_CORE:(c + 1) * N_PER_CORE, :])
        in_maps.append({"xx2": packed, "wall": wall, "g16": g16,
                        "cline": cline})

    res = run_bass_kernel_spmd(nc, in_maps, list(range(N_CORES)), trace=trace)
    out = np.concatenate([res.results[c]["out"] for c in range(N_CORES)], axis=0)
    if trace:
        kernel.last_exec_time_ns = res.exec_time_ns
        kernel.last_results = res
    return out


# revision 19
# speedup vs baseline: 1.0264x; 1.0264x over previous
"""Trainium2 Bass kernel for the MFA/MPPCA mixture log-likelihood problem.

Math: out[n,k] = PI[k] + logprob[n,k] with Sigma_k = A_k A_k^T + diag(D_k^2),
computed via Woodbury.  Everything involving only the small parameters
(MU, A, D, PI) is folded on the host into:

    out[n,k] = CONST[k] + x[n]·H[:,k] + (x[n]^2)·G[:,k] + sum_l (x[n]·Csc[:,k,l])^2

where (with iD = D^-2, B = iD*A, L = I + A^T B, iL = inv(L), R = chol(iL),
C0 = B R, e = R^T B^T MU):
    G   = -0.5 * iD^T                       (d, K)
    H   = (iD*MU)^T - C0 e                  (d, K)
    Csc = sqrt(0.5) * C0                    (d, K*l)
    CONST = PI - 0.5*(d log 2pi + logdet Sigma + MU^T iD MU) + 0.5 |e|^2

Device kernel (data-parallel over N on 8 cores, fp16 weights/inputs with
fp32 PSUM): x and x^2 are packed on the host into one fp16 stream per
128-sample tile (8 chunks of 128 features) so each tile needs exactly one
2KB-per-partition DMA line.  PE accumulates [H+G | Csc] projections into a
2-psum-bank tile (region a = H + components 0:32 at cols 0:384, region b =
components 32:64 at cols 512:832).  One ScalarE ACTIVATE squares all 640
factor projections (4-dim AP over both regions) into a fp32 [128, 64, 12]
buffer whose lane 10 is pre-filled with CONST[k] and lane 11 receives the
psum H+G block via a ScalarE copy; a single VectorE group-of-12 reduce then
emits the finished output row (q2 + CONST + H + G) directly.  DMAs are
batched 4 tiles at a time; a garbage-matmul warmup spans the initial DMA
window so the PE HAM clock-gate never starts the real work cold.
"""
import math
import numpy as np

N_TOTAL, K, D_FEAT, L_FAC = 131072, 64, 512, 10
N_CORES = 8
N_PER_CORE = N_TOTAL // N_CORES  # 16384

KHALF = K // 2                     # 32 components per psum region
ACOL = K + KHALF * L_FAC           # 384 = [H (0:64) | comps 0:32 (64:384)]
BCOL = KHALF * L_FAC               # 320 = comps 32:64
BOFF = 512                         # psum col offset of region b (bank 1)
TILES = N_PER_CORE // 128          # 128
DMA_BATCH = 4                      # sample tiles per input/output DMA


def host_prep(MU, A, D, PI):
    """Fold small-parameter math into matmul weights (float64 internally)."""
    MU64, A64, D64, PI64 = [np.asarray(v, np.float64) for v in (MU, A, D, PI)]
    Kc, d, l = A64.shape
    iD = D64 ** -2.0
    B = iD[..., None] * A64
    L = np.eye(l)[None] + np.einsum('kdl,kdm->klm', A64, B)
    sign, logdet_L = np.linalg.slogdet(L)
    log_det_Sigma = logdet_L - np.sum(np.log(iD), axis=1)
    iL = np.linalg.inv(L)
    R = np.linalg.cholesky(iL)                  # R @ R.T = iL
    C0 = np.einsum('kdl,klm->kdm', B, R)        # (K, d, l)
    bmu = np.einsum('kdl,kd->kl', B, MU64)
    e = np.einsum('klm,kl->km', R, bmu)         # (K, l)
    c1 = np.sum(iD * MU64 * MU64, axis=1)

    CONST = PI64 - 0.5 * (d * math.log(2.0 * math.pi) + log_det_Sigma + c1) \
        + 0.5 * np.sum(e * e, axis=1)
    G = (-0.5 * iD).T
    H = (iD * MU64 - np.einsum('kdm,km->kd', C0, e)).T
    Csc = (C0 * np.sqrt(0.5)).transpose(1, 0, 2).reshape(d, Kc * l)  # k-major

    wall = np.concatenate([H, Csc], axis=1).astype(np.float16)     # (d, 704)
    g16 = G.astype(np.float16)                                      # (d, K)
    cline = np.tile(CONST.astype(np.float32)[None, :], (128, 1))    # (128, K)
    return wall, g16, cline


def pack_core_input(xs):
    """xs: (n_per_core, 512) fp32 -> (TILES, 128*8*128) fp16 tile-major pack.

    Per tile t, partition p: 8 chunks of 128 contiguous fp16 values:
    chunks 0:4 = x[d = c*128+p, t*128:(t+1)*128], chunks 4:8 = x^2 likewise.
    """
    n = xs.shape[0]
    x16 = xs.T.astype(np.float16)                      # (512, n)
    x2 = (x16 * x16).astype(np.float16)                # exact squares of fp16 x
    xr = x16.reshape(4, 128, n // 128, 128).transpose(2, 1, 0, 3)
    x2r = x2.reshape(4, 128, n // 128, 128).transpose(2, 1, 0, 3)
    packed = np.concatenate([xr, x2r], axis=2)         # (t, 128, 8, 128)
    return np.ascontiguousarray(packed).reshape(n // 128, 128 * 8 * 128)


def build_nc(n_per_core=N_PER_CORE):
    """Build and compile the Bass module for one core (SPMD across 8)."""
    import concourse.bacc as bacc
    import concourse.tile as tile
    import concourse.mybir as mybir

    f32 = mybir.dt.float32
    f16 = mybir.dt.float16
    assert n_per_core % (128 * DMA_BATCH) == 0
    tiles = n_per_core // 128
    nbatch = tiles // DMA_BATCH

    nc = bacc.Bacc("TRN2", target_bir_lowering=False, debug=False,
                   enable_asserts=False, num_devices=N_CORES)
    xx2_dram = nc.dram_tensor("xx2", (tiles, 128 * 8 * 128), f16,
                              kind="ExternalInput")
    wall_dram = nc.dram_tensor("wall", (D_FEAT, K + K * L_FAC), f16,
                               kind="ExternalInput")
    g_dram = nc.dram_tensor("g16", (D_FEAT, K), f16, kind="ExternalInput")
    c_dram = nc.dram_tensor("cline", (128, K), f32, kind="ExternalInput")
    out_dram = nc.dram_tensor("out", (n_per_core, K), f32, kind="ExternalOutput")

    xx2_v = xx2_dram.ap().rearrange("t (p c j) -> t p c j", p=128, c=8)
    wall_v = wall_dram.ap().rearrange("(c p) m -> p c m", p=128)  # [128, 4, 704]
    g_v = g_dram.ap().rearrange("(c p) m -> p c m", p=128)        # [128, 4, 64]
    out_v = out_dram.ap().rearrange("(b u p) k -> b p u k", p=128, u=DMA_BATCH)

    with tile.TileContext(nc) as tc:
        with (
            tc.tile_pool(name="wpool", bufs=1) as wpool,
            tc.tile_pool(name="xpool", bufs=4) as xpool,
            tc.tile_pool(name="spool", bufs=3) as spool,
            tc.tile_pool(name="opool", bufs=2) as opool,
            tc.tile_pool(name="ppool", bufs=3, space="PSUM") as ppool,
            tc.tile_pool(name="wmpool", bufs=1, space="PSUM") as wmpool,
        ):
            # --- HAM warmup: keep PE busy at 2.4GHz while the first DMAs land.
            # memset on VectorE (fastest engine spin-up ~2.6us); the garbage
            # matmul results go to a scratch psum bank that is never read.
            warm = wpool.tile([128, 512], f16)
            nc.vector.memset(warm[:], 0.0)
            wpsum = wmpool.tile([128, 512], f32)
            NWARM = 24
            for j in range(NWARM):
                nc.tensor.matmul(wpsum[:], warm[:, 0:128], warm[:],
                                 start=(j == 0), stop=(j == NWARM - 1))

            wall_sb = wpool.tile([128, 4, K + K * L_FAC], f16)
            nc.sync.dma_start(out=wall_sb[:], in_=wall_v[:])
            g_sb = wpool.tile([128, 4, K], f16)
            nc.sync.dma_start(out=g_sb[:], in_=g_v[:])
            c_sb = wpool.tile([128, K], f32)
            nc.sync.dma_start(out=c_sb[:], in_=c_dram.ap())

            # ping-pong squares buffers [128, 64, 12] fp32: lanes 0:10 get the
            # squared projections, lane 10 holds CONST[k], lane 11 gets the
            # psum H+G block per tile -- the group reduce then emits the
            # finished output row directly
            sq0 = wpool.tile([128, K, 12], f32)
            sq1 = wpool.tile([128, K, 12], f32)
            for s in (sq0, sq1):
                nc.vector.tensor_copy(s[:, :, 10], c_sb[:])
            sqs = (sq0, sq1)

            for b in range(nbatch):
                xb = xpool.tile([128, DMA_BATCH, 8, 128], f16, tag="xb")
                nc.sync.dma_start(out=xb[:], in_=xx2_v[b * DMA_BATCH:
                                                       (b + 1) * DMA_BATCH]
                                  .rearrange("t p c j -> p t c j"))
                ob = opool.tile([128, DMA_BATCH, K], f32, tag="ob")
                for u in range(DMA_BATCH):
                    t = b * DMA_BATCH + u
                    psum = ppool.tile([128, 960], f32, tag="ps")

                    def mm_a(c, start, stop):
                        nc.tensor.matmul(psum[:, 0:ACOL], xb[:, u, c, :],
                                         wall_sb[:, c, 0:ACOL],
                                         start=start, stop=stop)

                    def mm_b(c, start, stop):
                        nc.tensor.matmul(psum[:, BOFF:BOFF + BCOL], xb[:, u, c, :],
                                         wall_sb[:, c, ACOL:ACOL + BCOL],
                                         start=start, stop=stop)

                    mm_a(0, True, False)
                    mm_b(0, True, False)
                    mm_a(1, False, False)
                    mm_b(1, False, False)
                    mm_a(2, False, False)
                    mm_b(2, False, False)
                    for c in range(4):
                        nc.tensor.matmul(psum[:, 0:K], xb[:, u, 4 + c, :],
                                         g_sb[:, c, :], start=False, stop=False,
                                         skip_group_check=True)
                    mm_a(3, False, True)
                    mm_b(3, False, True)

                    # one ACTIVATE squares both regions: AP [128, 2, 32, 10]
                    sq = sqs[t % 2]
                    # regions at cols 64 and 512 -> r-stride 448 cols
                    psq = psum[:, K:K + 2 * (BOFF - K)] \
                        .rearrange("p (r m) -> p r m", r=2)
                    nc.scalar.square(
                        sq[:].rearrange("p (r g) l -> p r g l", r=2)[:, :, :, 0:L_FAC],
                        psq[:, :, 0:KHALF * L_FAC]
                        .rearrange("p r (g t) -> p r g t", t=L_FAC))
                    nc.scalar.copy(sq[:, :, 11], psum[:, 0:K])

                    nc.vector.reduce_sum(ob[:, u, :], sq[:],
                                         axis=mybir.AxisListType.X)

                nc.sync.dma_start(out=out_v[b], in_=ob[:])

    nc.compile()
    return nc


_NC_CACHE = {}


def _get_nc(n_per_core=N_PER_CORE):
    if n_per_core not in _NC_CACHE:
        _NC_CACHE[n_per_core] = build_nc(n_per_core)
    return _NC_CACHE[n_per_core]


def _install_ntff_hook():
    """Provide the antenv.axon_hooks shim so trace=True can capture NTFFs."""
    import sys
    if "antenv.axon_hooks" in sys.modules:
        return
    import types
    import ctypes
    import contextlib

    so_path = "/opt/axon/libaxon_pjrt.so"
    lib = ctypes.CDLL(so_path)
    if not hasattr(lib, "axon_start_nrt_profile"):
        return
    lib.axon_start_nrt_profile.argtypes = [ctypes.POINTER(ctypes.c_int64), ctypes.c_size_t]
    lib.axon_start_nrt_profile.restype = ctypes.c_int64
    lib.axon_stop_nrt_profile.argtypes = [ctypes.c_char_p]
    lib.axon_stop_nrt_profile.restype = ctypes.c_int64

    @contextlib.contextmanager
    def _hook(output_dir, device_ids):
        import jax
        jax.devices()
        if device_ids:
            ids = (ctypes.c_int64 * len(device_ids))(*device_ids)
            rc = lib.axon_start_nrt_profile(ids, len(device_ids))
        else:
            rc = lib.axon_start_nrt_profile(None, 0)
        if rc != 0:
            raise RuntimeError(f"axon_start_nrt_profile rc={rc}")
        try:
            yield
        finally:
            n = lib.axon_stop_nrt_profile(str(output_dir).encode())
            print(f"ntff profile: {n} file(s) written to {output_dir}")

    mod = types.ModuleType("antenv.axon_hooks")
    mod.get_axon_ntff_profile_hook = lambda: _hook
    mod.set_axon_ntff_profile_hook = lambda h: None
    sys.modules["antenv.axon_hooks"] = mod


def kernel(x, MU, A, D, PI, trace=False):
    from concourse.bass_utils import run_bass_kernel_spmd
    if trace:
        try:
            _install_ntff_hook()
        except Exception as e:
            print(f"ntff hook install failed: {e}")
            trace = False

    x = np.asarray(x)
    wall, g16, cline = host_prep(MU, A, D, PI)
    nc = _get_nc()

    in_maps = []
    for c in range(N_CORES):
        packed = pack_core_input(x[c * N_PER_CORE:(c + 1) * N_PER_CORE, :])
        in_maps.append({"xx2": packed, "wall": wall, "g16": g16,
                        "cline": cline})

    res = run_bass_kernel_spmd(nc, in_maps, list(range(N_CORES)), trace=trace)
    out = np.concatenate([res.results[c]["out"] for c in range(N_CORES)], axis=0)
    if trace:
        kernel.last_exec_time_ns = res.exec_time_ns
        kernel.last_results = res
    return out


# revision 20
# speedup vs baseline: 1.0391x; 1.0123x over previous
"""Trainium2 Bass kernel for the MFA/MPPCA mixture log-likelihood problem.

Math: out[n,k] = PI[k] + logprob[n,k] with Sigma_k = A_k A_k^T + diag(D_k^2),
computed via Woodbury.  Everything involving only the small parameters
(MU, A, D, PI) is folded on the host into:

    out[n,k] = CONST[k] + x[n]·H[:,k] + (x[n]^2)·G[:,k] + sum_l (x[n]·Csc[:,k,l])^2

where (with iD = D^-2, B = iD*A, L = I + A^T B, iL = inv(L), R = chol(iL),
C0 = B R, e = R^T B^T MU):
    G   = -0.5 * iD^T                       (d, K)
    H   = (iD*MU)^T - C0 e                  (d, K)
    Csc = sqrt(0.5) * C0                    (d, K*l)
    CONST = PI - 0.5*(d log 2pi + logdet Sigma + MU^T iD MU) + 0.5 |e|^2

Device kernel (data-parallel over N on 8 cores, fp16 weights/inputs with
fp32 PSUM): x and x^2 are packed on the host into one fp16 stream per
128-sample tile (8 chunks of 128 features) so each tile needs exactly one
2KB-per-partition DMA line.  PE accumulates [H+G | Csc] projections into a
2-psum-bank tile (region a = H + components 0:32 at cols 0:384, region b =
components 32:64 at cols 512:832).  One ScalarE ACTIVATE squares all 640
factor projections (4-dim AP over both regions) into a fp32 [128, 64, 12]
buffer whose lane 10 is pre-filled with CONST[k] and lane 11 receives the
psum H+G block via a ScalarE copy; a single VectorE group-of-12 reduce then
emits the finished output row (q2 + CONST + H + G) directly.  DMAs are
batched 4 tiles at a time; a garbage-matmul warmup spans the initial DMA
window so the PE HAM clock-gate never starts the real work cold.
"""
import math
import numpy as np

N_TOTAL, K, D_FEAT, L_FAC = 131072, 64, 512, 10
N_CORES = 8
N_PER_CORE = N_TOTAL // N_CORES  # 16384

KHALF = K // 2                     # 32 components per psum region
ACOL = K + KHALF * L_FAC           # 384 = [H (0:64) | comps 0:32 (64:384)]
BCOL = KHALF * L_FAC               # 320 = comps 32:64
BOFF = 512                         # psum col offset of region b (bank 1)
TILES = N_PER_CORE // 128          # 128
DMA_BATCH = 4                      # sample tiles per input/output DMA


def host_prep(MU, A, D, PI):
    """Fold small-parameter math into matmul weights (float64 internally)."""
    MU64, A64, D64, PI64 = [np.asarray(v, np.float64) for v in (MU, A, D, PI)]
    Kc, d, l = A64.shape
    iD = D64 ** -2.0
    B = iD[..., None] * A64
    L = np.eye(l)[None] + np.einsum('kdl,kdm->klm', A64, B)
    sign, logdet_L = np.linalg.slogdet(L)
    log_det_Sigma = logdet_L - np.sum(np.log(iD), axis=1)
    iL = np.linalg.inv(L)
    R = np.linalg.cholesky(iL)                  # R @ R.T = iL
    C0 = np.einsum('kdl,klm->kdm', B, R)        # (K, d, l)
    bmu = np.einsum('kdl,kd->kl', B, MU64)
    e = np.einsum('klm,kl->km', R, bmu)         # (K, l)
    c1 = np.sum(iD * MU64 * MU64, axis=1)

    CONST = PI64 - 0.5 * (d * math.log(2.0 * math.pi) + log_det_Sigma + c1) \
        + 0.5 * np.sum(e * e, axis=1)
    G = (-0.5 * iD).T
    H = (iD * MU64 - np.einsum('kdm,km->kd', C0, e)).T
    Csc = (C0 * np.sqrt(0.5)).transpose(1, 0, 2).reshape(d, Kc * l)  # k-major

    wall = np.concatenate([H, Csc], axis=1).astype(np.float16)     # (d, 704)
    g16 = G.astype(np.float16)                                      # (d, K)
    cline = np.tile(CONST.astype(np.float32)[None, :], (128, 1))    # (128, K)
    return wall, g16, cline


def pack_core_input(xs):
    """xs: (n_per_core, 512) fp32 -> (TILES, 128*8*128) fp16 tile-major pack.

    Per tile t, partition p: 8 chunks of 128 contiguous fp16 values:
    chunks 0:4 = x[d = c*128+p, t*128:(t+1)*128], chunks 4:8 = x^2 likewise.
    """
    n = xs.shape[0]
    x16 = xs.T.astype(np.float16)                      # (512, n)
    x2 = (x16 * x16).astype(np.float16)                # exact squares of fp16 x
    xr = x16.reshape(4, 128, n // 128, 128).transpose(2, 1, 0, 3)
    x2r = x2.reshape(4, 128, n // 128, 128).transpose(2, 1, 0, 3)
    packed = np.concatenate([xr, x2r], axis=2)         # (t, 128, 8, 128)
    return np.ascontiguousarray(packed).reshape(n // 128, 128 * 8 * 128)


def build_nc(n_per_core=N_PER_CORE):
    """Build and compile the Bass module for one core (SPMD across 8)."""
    import concourse.bacc as bacc
    import concourse.tile as tile
    import concourse.mybir as mybir

    f32 = mybir.dt.float32
    f16 = mybir.dt.float16
    assert n_per_core % (128 * DMA_BATCH) == 0
    tiles = n_per_core // 128
    nbatch = tiles // DMA_BATCH

    nc = bacc.Bacc("TRN2", target_bir_lowering=False, debug=False,
                   enable_asserts=False, num_devices=N_CORES)
    xx2_dram = nc.dram_tensor("xx2", (tiles, 128 * 8 * 128), f16,
                              kind="ExternalInput")
    wall_dram = nc.dram_tensor("wall", (D_FEAT, K + K * L_FAC), f16,
                               kind="ExternalInput")
    g_dram = nc.dram_tensor("g16", (D_FEAT, K), f16, kind="ExternalInput")
    c_dram = nc.dram_tensor("cline", (128, K), f32, kind="ExternalInput")
    out_dram = nc.dram_tensor("out", (n_per_core, K), f32, kind="ExternalOutput")

    xx2_v = xx2_dram.ap().rearrange("t (p c j) -> t p c j", p=128, c=8)
    wall_v = wall_dram.ap().rearrange("(c p) m -> p c m", p=128)  # [128, 4, 704]
    g_v = g_dram.ap().rearrange("(c p) m -> p c m", p=128)        # [128, 4, 64]
    out_v = out_dram.ap().rearrange("(b u p) k -> b p u k", p=128, u=DMA_BATCH)

    with tile.TileContext(nc) as tc:
        with (
            tc.tile_pool(name="wpool", bufs=1) as wpool,
            tc.tile_pool(name="xpool", bufs=4) as xpool,
            tc.tile_pool(name="spool", bufs=3) as spool,
            tc.tile_pool(name="opool", bufs=2) as opool,
            tc.tile_pool(name="ppool", bufs=4, space="PSUM") as ppool,
        ):
            # --- HAM warmup: keep PE busy at 2.4GHz while the first DMAs land.
            # memset on VectorE; the garbage matmul results go into psum ring
            # slot 0, which the pipeline naturally recycles 4 tiles in.
            warm = wpool.tile([128, 512], f16)
            nc.vector.memset(warm[:], 0.0)
            wpsum = ppool.tile([128, 960], f32, tag="ps")
            NWARM = 15
            for j in range(NWARM):
                nc.tensor.matmul(wpsum[:, 0:512], warm[:, 0:128], warm[:],
                                 start=(j == 0), stop=(j == NWARM - 1))

            # DMA order on the sync queue: wall first (needed by the first
            # matmul), then the first half of batch 0, then the small params
            wall_sb = wpool.tile([128, 4, K + K * L_FAC], f16)
            nc.sync.dma_start(out=wall_sb[:], in_=wall_v[:])
            xb0 = xpool.tile([128, DMA_BATCH, 8, 128], f16, tag="xb")
            nc.sync.dma_start(out=xb0[:, 0:2], in_=xx2_v[0:2]
                              .rearrange("t p c j -> p t c j"))
            g_sb = wpool.tile([128, 4, K], f16)
            nc.sync.dma_start(out=g_sb[:], in_=g_v[:])
            c_sb = wpool.tile([128, K], f32)
            nc.sync.dma_start(out=c_sb[:], in_=c_dram.ap())
            nc.sync.dma_start(out=xb0[:, 2:4], in_=xx2_v[2:4]
                              .rearrange("t p c j -> p t c j"))

            # ping-pong squares buffers [128, 64, 12] fp32: lanes 0:10 get the
            # squared projections, lane 10 holds CONST[k], lane 11 gets the
            # psum H+G block per tile -- the group reduce then emits the
            # finished output row directly
            sq0 = wpool.tile([128, K, 12], f32)
            sq1 = wpool.tile([128, K, 12], f32)
            for s in (sq0, sq1):
                nc.vector.tensor_copy(s[:, :, 10], c_sb[:])
            sqs = (sq0, sq1)

            for b in range(nbatch):
                if b == 0:
                    xb = xb0
                else:
                    xb = xpool.tile([128, DMA_BATCH, 8, 128], f16, tag="xb")
                    nc.sync.dma_start(out=xb[:], in_=xx2_v[b * DMA_BATCH:
                                                           (b + 1) * DMA_BATCH]
                                      .rearrange("t p c j -> p t c j"))
                ob = opool.tile([128, DMA_BATCH, K], f32, tag="ob")
                for u in range(DMA_BATCH):
                    t = b * DMA_BATCH + u
                    psum = ppool.tile([128, 960], f32, tag="ps")

                    def mm_a(c, start, stop):
                        nc.tensor.matmul(psum[:, 0:ACOL], xb[:, u, c, :],
                                         wall_sb[:, c, 0:ACOL],
                                         start=start, stop=stop)

                    def mm_b(c, start, stop):
                        nc.tensor.matmul(psum[:, BOFF:BOFF + BCOL], xb[:, u, c, :],
                                         wall_sb[:, c, ACOL:ACOL + BCOL],
                                         start=start, stop=stop)

                    mm_a(0, True, False)
                    mm_b(0, True, False)
                    mm_a(1, False, False)
                    mm_b(1, False, False)
                    mm_a(2, False, False)
                    mm_b(2, False, False)
                    for c in range(4):
                        nc.tensor.matmul(psum[:, 0:K], xb[:, u, 4 + c, :],
                                         g_sb[:, c, :], start=False, stop=False,
                                         skip_group_check=True)
                    mm_a(3, False, True)
                    mm_b(3, False, True)

                    # one ACTIVATE squares both regions: AP [128, 2, 32, 10]
                    sq = sqs[t % 2]
                    # regions at cols 64 and 512 -> r-stride 448 cols
                    psq = psum[:, K:K + 2 * (BOFF - K)] \
                        .rearrange("p (r m) -> p r m", r=2)
                    nc.scalar.square(
                        sq[:].rearrange("p (r g) l -> p r g l", r=2)[:, :, :, 0:L_FAC],
                        psq[:, :, 0:KHALF * L_FAC]
                        .rearrange("p r (g t) -> p r g t", t=L_FAC))
                    nc.scalar.copy(sq[:, :, 11], psum[:, 0:K])

                    nc.vector.reduce_sum(ob[:, u, :], sq[:],
                                         axis=mybir.AxisListType.X)

                nc.sync.dma_start(out=out_v[b], in_=ob[:])

    nc.compile()
    return nc


_NC_CACHE = {}


def _get_nc(n_per_core=N_PER_CORE):
    if n_per_core not in _NC_CACHE:
        _NC_CACHE[n_per_core] = build_nc(n_per_core)
    return _NC_CACHE[n_per_core]


def _install_ntff_hook():
    """Provide the antenv.axon_hooks shim so trace=True can capture NTFFs."""
    import sys
    if "antenv.axon_hooks" in sys.modules:
        return
    import types
    import ctypes
    import contextlib

    so_path = "/opt/axon/libaxon_pjrt.so"
    lib = ctypes.CDLL(so_path)
    if not hasattr(lib, "axon_start_nrt_profile"):
        return
    lib.axon_start_nrt_profile.argtypes = [ctypes.POINTER(ctypes.c_int64), ctypes.c_size_t]
    lib.axon_start_nrt_profile.restype = ctypes.c_int64
    lib.axon_stop_nrt_profile.argtypes = [ctypes.c_char_p]
    lib.axon_stop_nrt_profile.restype = ctypes.c_int64

    @contextlib.contextmanager
    def _hook(output_dir, device_ids):
        import jax
        jax.devices()
        if device_ids:
            ids = (ctypes.c_int64 * len(device_ids))(*device_ids)
            rc = lib.axon_start_nrt_profile(ids, len(device_ids))
        else:
            rc = lib.axon_start_nrt_profile(None, 0)
        if rc != 0:
            raise RuntimeError(f"axon_start_nrt_profile rc={rc}")
        try:
            yield
        finally:
            n = lib.axon_stop_nrt_profile(str(output_dir).encode())
            print(f"ntff profile: {n} file(s) written to {output_dir}")

    mod = types.ModuleType("antenv.axon_hooks")
    mod.get_axon_ntff_profile_hook = lambda: _hook
    mod.set_axon_ntff_profile_hook = lambda h: None
    sys.modules["antenv.axon_hooks"] = mod


def kernel(x, MU, A, D, PI, trace=False):
    from concourse.bass_utils import run_bass_kernel_spmd
    if trace:
        try:
            _install_ntff_hook()
        except Exception as e:
            print(f"ntff hook install failed: {e}")
            trace = False

    x = np.asarray(x)
    wall, g16, cline = host_prep(MU, A, D, PI)
    nc = _get_nc()

    in_maps = []
    for c in range(N_CORES):
        packed = pack_core_input(x[c * N_PER_CORE:(c + 1) * N_PER_CORE, :])
        in_maps.append({"xx2": packed, "wall": wall, "g16": g16,
                        "cline": cline})

    res = run_bass_kernel_spmd(nc, in_maps, list(range(N_CORES)), trace=trace)
    out = np.concatenate([res.results[c]["out"] for c in range(N_CORES)], axis=0)
    if trace:
        kernel.last_exec_time_ns = res.exec_time_ns
        kernel.last_results = res
    return out
